# revision 1
# baseline (speedup 1.0000x reference)
"""Bass/Trainium2 kernel for nn_GAT_GCN (GAT -> GCN -> cross-attention -> MLP).

Sharding: 8 cores, each owns 128 consecutive graphs (batch is sorted, so a
contiguous node slab). Edges are assigned to the core owning their dst node,
so all segment reductions are core-local; cross-core data is an AllGather of
(a) small per-node attention tables and (b) the GAT output (needed globally
for the GCN's src-side gather).

Key algebraic restructurings (verified vs reference to ~7e-7 in numpy):
  * GAT aggregates in x-space (78-wide) instead of h-space (780-wide):
    out_h = (sum_e exp(a_e) x[src_e]) @ W_h, softmax denominator from an
    appended ones-column, bias folded via the s*b trick.
  * Segment sums are matmuls with 0/1 dst-masks built from iota==dstcol.
  * GCN normalization folded: rows pre-scaled by dinv (commutes with relu),
    dinv[dst] folded into the mask values.
  * Softmax max-subtraction dropped (logits are O(1); it cancels exactly).
"""
import os
import sys
import numpy as np

sys.path.insert(0, "/opt/trn_rl_repo")

KPHASE = int(os.environ.get("KPHASE", "9"))
KDBG = os.environ.get("KDBG", "0") == "1" 

N, E, B = 40000, 160000, 1024
FXD, H, DOUT, DP = 78, 10, 128, 256
NCORES, GPC = 8, 128          # graphs per core
P = 128
NPAD = 5376                   # padded nodes per core (42 * 128)
NW = NPAD // P                # node windows per core
NTOT = NCORES * NPAD          # padded global node space
HF = H * FXD                  # 780
HW1 = H * (FXD + 1)           # 790, z-psum width
SLOTS = 64                    # pooling slots per graph
GWIN = 8                      # windows per gather supertile
KC, NKC = 112, 7              # gcn/fcg1 K-chunk size/count (7*112=784>=780)
SRCW = FXD + 1 + H + 3        # 92: [x | 1 | asrc | pad]
DSTW = 12                     # [adst | pad]
DUMMY0 = NPAD  # pooling pad slots point at dummy -inf rows


# ----------------------------------------------------------------------------
# host preprocessing
# ----------------------------------------------------------------------------

def _host_prep(inputs):
    x = np.ascontiguousarray(np.asarray(inputs["x"], dtype=np.float32))
    edge_index = np.asarray(inputs["edge_index"]).astype(np.int64)
    batch = np.asarray(inputs["batch"]).astype(np.int64)
    pvec = np.asarray(inputs["protein_vec"], dtype=np.float32)

    src = np.concatenate([edge_index[0], np.arange(N, dtype=np.int64)])
    dst = np.concatenate([edge_index[1], np.arange(N, dtype=np.int64)])
    order = np.argsort(dst, kind="stable")
    src, dst = src[order], dst[order]

    node_core = batch // GPC
    n0 = np.searchsorted(node_core, np.arange(NCORES))
    n1 = np.searchsorted(node_core, np.arange(NCORES), side="right")
    ncount = n1 - n0
    assert ncount.max() <= NPAD, ncount.max()
    pad_id = np.zeros(N, np.int64)
    for c in range(NCORES):
        pad_id[n0[c]:n1[c]] = c * NPAD + np.arange(ncount[c])

    deg = np.bincount(dst, minlength=N).astype(np.float32)

    # per (core, window) edge lists; shared tile schedule = max over cores
    e_start = np.searchsorted(dst, n0)
    e_end = np.searchsorted(dst, n1)
    win_edges = [[None] * NW for _ in range(NCORES)]
    tiles_per_win = np.zeros(NW, np.int64)
    for c in range(NCORES):
        es, ed = src[e_start[c]:e_end[c]], dst[e_start[c]:e_end[c]]
        loc = ed - n0[c]
        wbound = np.searchsorted(loc, np.arange(NW + 1) * P)
        for w in range(NW):
            a, b = int(wbound[w]), int(wbound[w + 1])
            win_edges[c][w] = (es[a:b], loc[a:b])
            tiles_per_win[w] = max(tiles_per_win[w], (b - a + P - 1) // P)
    ttot = int(tiles_per_win.sum())
    tile_start = np.zeros(NW + 1, np.int64)
    for w in range(NW):
        tile_start[w + 1] = tile_start[w] + tiles_per_win[w]

    groups = []
    for g0 in range(0, NW, GWIN):
        g1 = min(g0 + GWIN, NW)
        groups.append((g0, g1, int(tile_start[g0]), int(tile_start[g1])))

    meta = dict(ttot=ttot, tiles_per_win=[int(v) for v in tiles_per_win],
                tile_start=[int(v) for v in tile_start], groups=groups)

    # weight folds (host: tiny)
    gat_w = np.asarray(inputs["gat_w"], np.float32)
    a_s = np.asarray(inputs["gat_att_src"], np.float32)
    a_d = np.asarray(inputs["gat_att_dst"], np.float32)
    gat_b = np.asarray(inputs["gat_b"], np.float32)
    Vsd = np.zeros((FXD, 2 * H), np.float32)
    Wcat = np.zeros((FXD + 1, HF), np.float32)
    for h in range(H):
        Wh = gat_w[:, h * FXD:(h + 1) * FXD]
        Vsd[:, h] = Wh @ a_s[h]
        Vsd[:, H + h] = Wh @ a_d[h]
        Wcat[:FXD, h * FXD:(h + 1) * FXD] = Wh
        Wcat[FXD, h * FXD:(h + 1) * FXD] = gat_b[h * FXD:(h + 1) * FXD]

    def kchunk(wm, kc, nk, ncols):           # [K, ncols] -> [kc, nk*ncols]
        wp = np.zeros((kc * nk, ncols), np.float32)
        wp[:wm.shape[0]] = wm
        out = np.zeros((kc, nk * ncols), np.float32)
        for k in range(nk):
            out[:, k * ncols:(k + 1) * ncols] = wp[k * kc:(k + 1) * kc]
        return out

    shared = {
        "v_sd": Vsd, "wcat": Wcat,
        "gcnw": kchunk(np.asarray(inputs["gcn_w"], np.float32), KC, NKC, HF),
        "gcnb": np.asarray(inputs["gcn_b"], np.float32)[None, :],
        "fcg1w": kchunk(np.asarray(inputs["fcg1_w"], np.float32), KC, NKC, DOUT),
        "fcg1b": np.asarray(inputs["fcg1_b"], np.float32)[:, None],
        "pfcw": kchunk(np.asarray(inputs["pfc_w"], np.float32), P, 2, DP),
        "pfcb": np.asarray(inputs["pfc_b"], np.float32)[None, :],
        "qw": np.asarray(inputs["q_w"], np.float32),
        "qb": np.asarray(inputs["q_b"], np.float32)[None, :],
        "kw": kchunk(np.asarray(inputs["k_w"], np.float32), P, 2, P),
        "kbr": np.asarray(inputs["k_b"], np.float32)[None, :],
        "vw": kchunk(np.asarray(inputs["v_w"], np.float32), P, 2, P),
        "vbr": np.asarray(inputs["v_b"], np.float32)[None, :],
        "fc1w": kchunk(np.asarray(inputs["fc1_w"], np.float32), P, 3, 1024),
        "fc1b": np.asarray(inputs["fc1_b"], np.float32)[None, :],
        "fc2w": kchunk(np.asarray(inputs["fc2_w"], np.float32), P, 8, 512),
        "fc2b": np.asarray(inputs["fc2_b"], np.float32)[None, :],
        "outw": kchunk(np.asarray(inputs["out_w"], np.float32), P, 4, 1),
        "outb": np.asarray(inputs["out_b"], np.float32)[None, :],
        "iota": np.tile(np.arange(P, dtype=np.float32), (P, 1)),
        "ident": np.eye(P, dtype=np.float32),
    }

    in_maps = []
    for c in range(NCORES):
        nloc = int(ncount[c])
        src_idx = np.zeros((P, ttot), np.int32)
        dstcol = np.full((P, ttot), 999.0, np.float32)
        dinv_e = np.zeros((P, ttot), np.float32)
        for w in range(NW):
            es, loc = win_edges[c][w]
            ne = len(es)
            for j in range(int(tiles_per_win[w])):
                t = int(tile_start[w]) + j
                a, b = j * P, min((j + 1) * P, ne)
                if a >= ne:
                    continue
                m = b - a
                src_idx[:m, t] = pad_id[es[a:b]]
                gdst = loc[a:b] + n0[c]
                dstcol[:m, t] = (loc[a:b] - w * P).astype(np.float32)
                dinv_e[:m, t] = deg[gdst] ** -0.5

        x_slab = np.zeros((NPAD, FXD), np.float32)
        x_slab[:nloc] = x[n0[c]:n1[c]]

        # node-major [P, NW] arrays: node w*P+p at [p, w]
        deg_flat = np.ones(NPAD, np.float32)
        deg_flat[:nloc] = deg[n0[c]:n1[c]]
        deg_local = deg_flat.reshape(NW, P).T.copy()
        bc_flat = np.full(NPAD, -1.0, np.float32)
        bc_flat[:nloc] = (batch[n0[c]:n1[c]] - c * GPC).astype(np.float32)
        batchcol = bc_flat.reshape(NW, P).T.copy()

        # pooling slots: flat slot g*SLOTS+s -> dense-local node idx
        flat = np.arange(GPC * SLOTS, dtype=np.int64) % P + DUMMY0
        bl = batch[n0[c]:n1[c]] - c * GPC
        gstart = np.searchsorted(bl, np.arange(GPC + 1))
        for g in range(GPC):
            a, b = int(gstart[g]), int(gstart[g + 1])
            m = min(b - a, SLOTS)
            flat[g * SLOTS:g * SLOTS + m] = np.arange(a, a + m)
        # gather tile tp reads slots [tp*P,(tp+1)*P): idx_pool[p, tp]
        idx_pool = flat.reshape(GPC * SLOTS // P, P).T.astype(np.int32).copy()

        pv_slab = pvec[c * GPC:(c + 1) * GPC, 0, :]          # [128, 256]
        pvT = np.zeros((P, 2 * P), np.float32)
        for k in range(2):
            pvT[:, k * P:(k + 1) * P] = pv_slab[:, k * P:(k + 1) * P].T

        im = {"x_slab": x_slab, "src_idx": src_idx,
              "dstcol": dstcol, "dinv_e": dinv_e, "deg_local": deg_local,
              "batchcol": batchcol, "idx_pool": idx_pool, "pvt": pvT}
        im.update(shared)
        in_maps.append({k: np.ascontiguousarray(v) for k, v in im.items()})

    return in_maps, meta


# ----------------------------------------------------------------------------
# device program
# ----------------------------------------------------------------------------

def _build_program(meta):
    from concourse import bass, bacc, mybir, tile

    f32 = mybir.dt.float32
    f32r = mybir.dt.float32r
    i32 = mybir.dt.int32
    AF = mybir.ActivationFunctionType
    OP = mybir.AluOpType
    AX = mybir.AxisListType
    IOA = bass.IndirectOffsetOnAxis

    ttot = meta["ttot"]
    tpw = meta["tiles_per_win"]
    tstart = meta["tile_start"]
    groups = meta["groups"]

    nc = bacc.Bacc("TRN2", target_bir_lowering=False, debug=False,
                   enable_asserts=False, num_devices=NCORES)

    def din(name, shape, dt=f32):
        return nc.dram_tensor(name, list(shape), dt, kind="ExternalInput").ap()

    x_slab = din("x_slab", (NPAD, FXD))
    d_in = {
        "src_idx": din("src_idx", (P, ttot), i32),
        "dstcol": din("dstcol", (P, ttot)),
        "dinv_e": din("dinv_e", (P, ttot)),
        "deg_local": din("deg_local", (P, NW)),
        "batchcol": din("batchcol", (P, NW)),
        "idx_pool": din("idx_pool", (P, SLOTS), i32),
        "pvt": din("pvt", (P, 2 * P)),
        "v_sd": din("v_sd", (FXD, 2 * H)),
        "wcat": din("wcat", (FXD + 1, HF)),
        "pfcw": din("pfcw", (P, 2 * DP)),
        "pfcb": din("pfcb", (1, DP)),
        "qw": din("qw", (P, P)),
        "qb": din("qb", (1, P)),
        "kw": din("kw", (P, 2 * P)),
        "kbr": din("kbr", (1, P)),
        "vw": din("vw", (P, 2 * P)),
        "vbr": din("vbr", (1, P)),
        "iota": din("iota", (P, P)),
        "ident": din("ident", (P, P)),
    }
    gcnw_d = din("gcnw", (KC, NKC * HF))
    gcnb_d = din("gcnb", (1, HF))
    fcg1w_d = din("fcg1w", (KC, NKC * DOUT))
    fcg1b_d = din("fcg1b", (P, 1))
    fc1w_d = din("fc1w", (P, 3 * 1024))
    fc1b_d = din("fc1b", (1, 1024))
    fc2w_d = din("fc2w", (P, 8 * 512))
    fc2b_d = din("fc2b", (1, 512))
    outw_d = din("outw", (P, 4))
    outb_d = din("outb", (1, 1))

    y = nc.dram_tensor("y", [GPC, 1], f32, kind="ExternalOutput").ap()
    dbg_h = nc.dram_tensor("dbg_h", [NPAD, HF], f32, kind="ExternalOutput").ap()
    dbg_m = nc.dram_tensor("dbg_m", [P, NPAD], f32, kind="ExternalOutput").ap()
    dbg_s = nc.dram_tensor("dbg_s", [P, 300], f32, kind="ExternalOutput").ap()

    with tile.TileContext(nc) as tc:
      with tc.tile_pool(name="cst", bufs=1) as cst, \
           tc.tile_pool(name="dram", bufs=1, space="DRAM") as dram:

        def cload(pool, ap_):
            t = pool.tile(list(ap_.shape), ap_.dtype, tag=ap_.tensor.name)
            nc.sync.dma_start(out=t[:], in_=ap_)
            return t

        C = {k: cload(cst, v) for k, v in d_in.items()}
        iota, ident = C["iota"], C["ident"]

        ones = cst.tile([1, P], f32)
        nc.vector.memset(ones[:], 1.0)
        degs = cst.tile([P, NW], f32)
        dinv_all = cst.tile([P, NW], f32)
        nc.scalar.sqrt(degs[:], C["deg_local"][:])
        nc.vector.reciprocal(dinv_all[:], degs[:])

        ag_src_in = dram.tile([NPAD, SRCW], f32)
        table_src = dram.tile([NTOT, SRCW], f32)
        ag_h_in = dram.tile([NPAD, HF], f32)
        h2s_full = dram.tile([NTOT, HF], f32)
        att_dense = dram.tile([NPAD + P, DOUT], f32)
        adst_all = cst.tile([P, NW * H], f32)

        rg = [list(range(NCORES))]

        def mm(out, lhsT, rhs, start, stop, rdt=None):
            if rdt is not None:
                lhsT = lhsT.bitcast(rdt)
                rhs = rhs.bitcast(rdt)
            nc.tensor.matmul(out, lhsT, rhs, start=start, stop=stop)

        # ------------------------------------------------------------------
        # Phase A: build [x|1|asrc] / [adst] tables, AllGather them
        # ------------------------------------------------------------------
        with tc.tile_pool(name="pha", bufs=3) as wk, \
             tc.tile_pool(name="pha_ps", bufs=2, space="PSUM") as ps:
            for w in range(NW):
                xw = wk.tile([P, FXD], f32, tag="xw")
                nc.sync.dma_start(out=xw[:], in_=x_slab[w * P:(w + 1) * P, :])
                pst = ps.tile([FXD, P], f32, tag="xt")
                nc.tensor.transpose(pst[:], xw[:], ident[:])
                xT = wk.tile([FXD, P], f32, tag="xT")
                nc.vector.tensor_copy(xT[:], pst[:])
                pav = ps.tile([P, 2 * H], f32, tag="av")
                mm(pav[:], xT[:], C["v_sd"][:], True, True)
                asb = wk.tile([P, SRCW], f32, tag="asb")
                nc.vector.tensor_copy(asb[:, 0:FXD], xw[:])
                nc.vector.memset(asb[:, FXD:FXD + 1], 1.0)
                nc.vector.tensor_copy(asb[:, FXD + 1:FXD + 1 + H], pav[:, 0:H])
                nc.vector.memset(asb[:, FXD + 1 + H:SRCW], 0.0)
                nc.vector.tensor_copy(adst_all[:, w * H:(w + 1) * H],
                                      pav[:, H:2 * H])
                nc.sync.dma_start(out=ag_src_in[w * P:(w + 1) * P, :], in_=asb[:])

        nc.gpsimd.collective_compute(
            "AllGather", OP.bypass, ins=[ag_src_in.opt()],
            outs=[table_src.opt()], replica_groups=rg)

        # ------------------------------------------------------------------
        # Phase B: GAT
        # ------------------------------------------------------------------
        with tc.tile_pool(name="phb_g", bufs=2) as gp, \
             tc.tile_pool(name="phb_w", bufs=3) as wk, \
             tc.tile_pool(name="phb_z", bufs=2, space="PSUM") as psz, \
             tc.tile_pool(name="phb_t", bufs=2, space="PSUM") as pstp, \
             tc.tile_pool(name="phb_u", bufs=2, space="PSUM") as psu:
            for w in (range(NW) if KPHASE >= 2 else []):
                    nt = tpw[w]
                    if nt == 0:
                        continue
                    pz = psz.tile([P, HW1], f32, tag="pz")
                    for j in range(nt):
                        t = tstart[w] + j
                        xg = gp.tile([P, SRCW], f32, tag="xg")
                        nc.gpsimd.indirect_dma_start(
                            out=xg[:], out_offset=None, in_=table_src[:],
                            in_offset=IOA(ap=C["src_idx"][:, t:t + 1], axis=0))
                        msk = wk.tile([P, P], f32, tag="msk")
                        nc.vector.tensor_scalar(
                            out=msk[:], in0=iota[:],
                            scalar1=C["dstcol"][:, t:t + 1],
                            scalar2=None, op0=OP.is_equal)
                        pmt = pstp.tile([P, P], f32, tag="pzt")
                        nc.tensor.transpose(pmt[:], msk[:], ident[:])
                        mskT = wk.tile([P, P], f32, tag="mskT")
                        nc.scalar.copy(mskT[:], pmt[:])
                        pad_e = psu.tile([P, H], f32, tag="pu")
                        mm(pad_e[:], mskT[:],
                           adst_all[:, w * H:(w + 1) * H], True, True)
                        al = wk.tile([P, H], f32, tag="al")
                        nc.vector.tensor_tensor(
                            out=al[:], in0=xg[:, FXD + 1:FXD + 1 + H],
                            in1=pad_e[:], op=OP.add)
                        lr = wk.tile([P, H], f32, tag="lr")
                        nc.vector.scalar_tensor_tensor(
                            out=lr[:], in0=al[:], scalar=0.2, in1=al[:],
                            op0=OP.mult, op1=OP.max)
                        ea = wk.tile([P, H], f32, tag="ea")
                        nc.scalar.activation(ea[:], lr[:], AF.Exp)
                        xgs = wk.tile([P, H, FXD + 1], f32, tag="xgs")
                        nc.vector.tensor_tensor(
                            out=xgs[:],
                            in0=xg[:, 0:FXD + 1].unsqueeze(1)
                                .broadcast_to([P, H, FXD + 1]),
                            in1=ea[:].unsqueeze(2)
                                .broadcast_to([P, H, FXD + 1]),
                            op=OP.mult)
                        xf = xgs[:].rearrange("p a b -> p (a b)")
                        mm(pz[:, 0:512], msk[:], xf[:, 0:512],
                           j == 0, j == nt - 1)
                        mm(pz[:, 512:HW1], msk[:], xf[:, 512:HW1],
                           j == 0, j == nt - 1)
                    zsb = wk.tile([P, HW1], f32, tag="zsb")
                    nc.scalar.copy(zsb[:], pz[:])
                    stmp = wk.tile([P, H], f32, tag="stmp")
                    nc.vector.tensor_scalar(
                        out=stmp[:], in0=zsb[:, FXD::FXD + 1], scalar1=1e-30,
                        scalar2=None, op0=OP.add)
                    sinv = wk.tile([P, H], f32, tag="sinv")
                    nc.vector.reciprocal(sinv[:], stmp[:])
                    sc = wk.tile([P, H], f32, tag="sc")
                    nc.vector.tensor_scalar_mul(sc[:], sinv[:],
                                                dinv_all[:, w:w + 1])
                    h2sb = wk.tile([P, HF], f32, tag="h2sb")
                    for h in range(H):
                        pzt = pstp.tile([FXD + 1, P], f32, tag="pzt")
                        nc.tensor.transpose(
                            pzt[:], zsb[:, h * (FXD + 1):(h + 1) * (FXD + 1)],
                            ident[:])
                        zt = wk.tile([FXD + 1, P], f32, tag="zt")
                        nc.vector.tensor_copy(zt[:], pzt[:])
                        pu = psu.tile([P, FXD], f32, tag="pu")
                        mm(pu[:], zt[:], C["wcat"][:, h * FXD:(h + 1) * FXD],
                           True, True)
                        nc.scalar.activation(
                            h2sb[:, h * FXD:(h + 1) * FXD], pu[:], AF.Relu,
                            scale=sc[:, h:h + 1])
                    nc.sync.dma_start(out=ag_h_in[w * P:(w + 1) * P, :],
                                      in_=h2sb[:])
                    if KDBG:
                        nc.sync.dma_start(out=dbg_h[w * P:(w + 1) * P, :],
                                          in_=h2sb[:])

        nc.gpsimd.collective_compute(
            "AllGather", OP.bypass, ins=[ag_h_in.opt()],
            outs=[h2s_full.opt()], replica_groups=rg)

        # persistent attention-phase buffers
        with tc.tile_pool(name="att", bufs=1) as att:
            dnT = att.tile([P, NPAD], f32)
            e_all = att.tile([P, NW], f32)
            nc.vector.memset(dnT[:], 0.0)
            nc.vector.memset(e_all[:], 0.0)

            # protein path (independent of the graph phases)
            pv_sb = att.tile([P, DP], f32)
            pvt2 = att.tile([P, 2 * P], f32)
            k_sb = att.tile([P, P], f32)
            v_sb = att.tile([P, P], f32)
            with tc.tile_pool(name="prot_ps", bufs=2, space="PSUM") as ps:
                ppv = ps.tile([P, DP], f32, tag="ppv")
                for k in range(2):
                    mm(ppv[:], C["pvt"][:, k * P:(k + 1) * P],
                       C["pfcw"][:, k * DP:(k + 1) * DP], k == 0, False)
                mm(ppv[:], ones[:], C["pfcb"][:], False, True)
                nc.scalar.activation(pv_sb[:], ppv[:], AF.Relu)
                for k in range(2):
                    pt = ps.tile([P, P], f32, tag="pt")
                    nc.tensor.transpose(pt[:], pv_sb[:, k * P:(k + 1) * P],
                                        ident[:])
                    nc.vector.tensor_copy(pvt2[:, k * P:(k + 1) * P], pt[:])
                for dst_t, wname, bname in ((k_sb, "kw", "kbr"),
                                            (v_sb, "vw", "vbr")):
                    pk = ps.tile([P, P], f32, tag="pk")
                    for k in range(2):
                        mm(pk[:], pvt2[:, k * P:(k + 1) * P],
                           C[wname][:, k * P:(k + 1) * P], k == 0, False)
                    mm(pk[:], ones[:], C[bname][:], False, True)
                    nc.vector.tensor_copy(dst_t[:], pk[:])

            # --------------------------------------------------------------
            # Phase C: GCN + fcg1 (writes dnT)
            # --------------------------------------------------------------
            with tc.tile_pool(name="phc_c", bufs=1) as cc, \
                 tc.tile_pool(name="phc_g", bufs=2) as gp, \
                 tc.tile_pool(name="phc_w", bufs=2) as wk, \
                 tc.tile_pool(name="phc_a", bufs=1, space="PSUM") as psa, \
                 tc.tile_pool(name="phc_t", bufs=2, space="PSUM") as pstp, \
                 tc.tile_pool(name="phc_d", bufs=1, space="PSUM") as psd:
                gcnw = cload(cc, gcnw_d)
                gcnb = cload(cc, gcnb_d)
                fcg1w = cload(cc, fcg1w_d)
                fcg1b = cload(cc, fcg1b_d)
                for w in (range(NW) if KPHASE >= 3 else []):
                    nt = tpw[w]
                    if nt == 0:
                        continue
                    t0 = tstart[w]
                    pagg = psa.tile([P, HF], f32, tag="pagg")
                    for j in range(nt):
                        t = t0 + j
                        hg = gp.tile([P, HF], f32, tag="hg")
                        nc.gpsimd.indirect_dma_start(
                            out=hg[:], out_offset=None, in_=h2s_full[:],
                            in_offset=IOA(ap=C["src_idx"][:, t:t + 1], axis=0))
                        mskd = wk.tile([P, P], f32, tag="mskd")
                        nc.vector.tensor_scalar(
                            out=mskd[:], in0=iota[:],
                            scalar1=C["dstcol"][:, t:t + 1],
                            scalar2=C["dinv_e"][:, t:t + 1],
                            op0=OP.is_equal, op1=OP.mult)
                        mm(pagg[:, 0:512], mskd[:], hg[:, 0:512],
                           j == 0, j == nt - 1)
                        mm(pagg[:, 512:HF], mskd[:], hg[:, 512:HF],
                           j == 0, j == nt - 1)
                    asb = wk.tile([P, KC * NKC], f32, tag="asb")
                    nc.scalar.copy(asb[:, 0:HF], pagg[:])
                    nc.vector.memset(asb[:, HF:KC * NKC], 0.0)
                    aT = wk.tile([KC, NKC * P], f32, tag="aT")
                    for k in range(NKC):
                        ptr = pstp.tile([KC, P], f32, tag="ptr")
                        nc.tensor.transpose(ptr[:],
                                            asb[:, k * KC:(k + 1) * KC],
                                            ident[:])
                        nc.vector.tensor_copy(aT[:, k * P:(k + 1) * P],
                                              ptr[:])
                    pdr = psd.tile([P, HF], f32, tag="pdr")
                    for k in range(NKC):
                        mm(pdr[:, 0:512], aT[:, k * P:(k + 1) * P],
                           gcnw[:, k * HF:k * HF + 512], k == 0, False)
                        mm(pdr[:, 512:HF], aT[:, k * P:(k + 1) * P],
                           gcnw[:, k * HF + 512:(k + 1) * HF],
                           k == 0, False)
                    mm(pdr[:, 0:512], ones[:], gcnb[:, 0:512],
                       False, True)
                    mm(pdr[:, 512:HF], ones[:], gcnb[:, 512:HF],
                       False, True)
                    drg = wk.tile([P, KC * NKC], f32, tag="drg")
                    nc.scalar.activation(drg[:, 0:HF], pdr[:], AF.Relu)
                    nc.vector.memset(drg[:, HF:KC * NKC], 0.0)
                    drT = wk.tile([KC, NKC * P], f32, tag="drT")
                    for k in range(NKC):
                        ptr = pstp.tile([KC, P], f32, tag="ptr")
                        nc.tensor.transpose(ptr[:],
                                            drg[:, k * KC:(k + 1) * KC],
                                            ident[:])
                        nc.vector.tensor_copy(drT[:, k * P:(k + 1) * P],
                                              ptr[:])
                    pdn = psd.tile([P, P], f32, tag="pdn")
                    for k in range(NKC):
                        mm(pdn[:], fcg1w[:, k * P:(k + 1) * P],
                           drT[:, k * P:(k + 1) * P], k == 0, k == NKC - 1)
                    nc.scalar.activation(dnT[:, w * P:(w + 1) * P], pdn[:],
                                         AF.Relu, bias=fcg1b[:])

            # --------------------------------------------------------------
            # Phase D: cross attention + pooling + MLP
            # --------------------------------------------------------------
            if KDBG and KPHASE >= 3:
                nc.sync.dma_start(out=dbg_m, in_=dnT[:])
            with tc.tile_pool(name="phd_c", bufs=1) as cc, \
                 tc.tile_pool(name="phd_w", bufs=3) as wk, \
                 tc.tile_pool(name="phd_ps", bufs=2, space="PSUM") as ps:
                fc1w = cload(cc, fc1w_d)
                fc1b = cload(cc, fc1b_d)
                fc2w = cload(cc, fc2w_d)
                fc2b = cload(cc, fc2b_d)
                outw = cload(cc, outw_d)
                outb = cload(cc, outb_d)

                def build_g01(b):
                    g01t = wk.tile([P, P], f32, tag="g01t")
                    nc.vector.tensor_scalar(
                        out=g01t[:], in0=iota[:],
                        scalar1=C["batchcol"][:, b:b + 1],
                        scalar2=None, op0=OP.is_equal)
                    return g01t

                def build_g01_gmajor(b):
                    g01t = build_g01(b)
                    pg = ps.tile([P, P], f32, tag="s")
                    nc.tensor.transpose(pg[:], g01t[:], ident[:])
                    gsb = wk.tile([P, P], f32, tag="gsb")
                    nc.vector.tensor_copy(gsb[:], pg[:])
                    return gsb

                # pass 1: scores
                for b in (range(NW) if KPHASE >= 4 else []):
                    gsb = build_g01_gmajor(b)
                    pq = ps.tile([P, P], f32, tag="pq")
                    mm(pq[:], dnT[:, b * P:(b + 1) * P], C["qw"][:],
                       True, False)
                    mm(pq[:], ones[:], C["qb"][:], False, True)
                    pkb = ps.tile([P, P], f32, tag="s")
                    mm(pkb[:], gsb[:], k_sb[:], True, True)
                    kbs = wk.tile([P, P], f32, tag="kbs")
                    nc.scalar.copy(kbs[:], pkb[:])
                    qkb = wk.tile([P, P], f32, tag="qkb")
                    nc.vector.tensor_tensor(out=qkb[:], in0=pq[:],
                                            in1=kbs[:], op=OP.mult)
                    scb = wk.tile([P, 1], f32, tag="scb")
                    nc.vector.reduce_sum(out=scb[:], in_=qkb[:], axis=AX.X)
                    nc.scalar.activation(e_all[:, b:b + 1], scb[:], AF.Exp,
                                         scale=float(1.0 / np.sqrt(128.0)))
                if KDBG and KPHASE >= 4:
                    nc.sync.dma_start(out=dbg_s[:, 0:NW], in_=e_all[:])
                if KPHASE < 5:
                    zout = wk.tile([P, 1], f32, tag="zout")
                    nc.vector.memset(zout[:], 0.0)
                    nc.sync.dma_start(out=y, in_=zout[:])
                else:
                    # graph-softmax sums
                    pss = ps.tile([P, 1], f32, tag="acc")
                    for b in (range(NW) if KPHASE >= 4 else []):
                        g01t = build_g01(b)
                        mm(pss[:], g01t[:], e_all[:, b:b + 1], b == 0, b == NW - 1)
                    ssum = wk.tile([P, 1], f32, tag="ssum")
                    nc.vector.tensor_scalar(out=ssum[:], in0=pss[:],
                                            scalar1=1e-30, scalar2=None,
                                            op0=OP.add)
                    sinv_g = att.tile([P, 1], f32)
                    nc.vector.reciprocal(sinv_g[:], ssum[:])
                    ninf = wk.tile([P, DOUT], f32, tag="ninf")
                    nc.vector.memset(ninf[:], -1e30)
                    nc.sync.dma_start(out=att_dense[NPAD:NPAD + P, :], in_=ninf[:])
                    # pass 2: attended -> att_dense
                    for b in range(NW):
                        gsb = build_g01_gmajor(b)
                        psn = ps.tile([P, 1], f32, tag="acc")
                        mm(psn[:], gsb[:], sinv_g[:], True, True)
                        atn = wk.tile([P, 1], f32, tag="atn")
                        nc.vector.tensor_tensor(out=atn[:], in0=e_all[:, b:b + 1],
                                                in1=psn[:], op=OP.mult)
                        pvb = ps.tile([P, P], f32, tag="s")
                        mm(pvb[:], gsb[:], v_sb[:], True, True)
                        va = wk.tile([P, P], f32, tag="va")
                        nc.vector.tensor_scalar_mul(va[:], pvb[:], atn[:])
                        pdt = ps.tile([P, P], f32, tag="s")
                        nc.tensor.transpose(pdt[:], dnT[:, b * P:(b + 1) * P],
                                            ident[:])
                        attb = wk.tile([P, P], f32, tag="attb")
                        nc.vector.tensor_tensor(out=attb[:], in0=pdt[:],
                                                in1=va[:], op=OP.add)
                        nc.sync.dma_start(out=att_dense[b * P:(b + 1) * P, :],
                                          in_=attb[:])
                    if KPHASE >= 6:
                        # pooling
                        pooledT = att.tile([P, P], f32)
                        NPT = GPC * SLOTS // P
                        for tp in range(NPT):
                                pgt = wk.tile([P, DOUT], f32, tag="pgt")
                                nc.vector.memset(pgt[:], -1e30)
                                nc.gpsimd.indirect_dma_start(
                                    out=pgt[:], out_offset=None, in_=att_dense[:],
                                    in_offset=IOA(ap=C["idx_pool"][:, tp:tp + 1],
                                                  axis=0))
                                ppt = ps.tile([P, P], f32, tag="s")
                                nc.tensor.transpose(ppt[:], pgt[:], ident[:])
                                g0 = tp * P // SLOTS
                                nc.vector.reduce_max(out=pooledT[:, g0:g0 + 1],
                                                     in_=ppt[:, 0:SLOTS], axis=AX.X)
                                nc.vector.reduce_max(out=pooledT[:, g0 + 1:g0 + 2],
                                                     in_=ppt[:, SLOTS:P], axis=AX.X)
                        pmask = wk.tile([P, P], f32, tag="pmask")
                        nc.vector.tensor_scalar(out=pmask[:], in0=pooledT[:],
                                                scalar1=-1e29, scalar2=None,
                                                op0=OP.is_ge)
                        pooled0 = att.tile([P, P], f32)
                        nc.vector.tensor_tensor(out=pooled0[:], in0=pooledT[:],
                                                in1=pmask[:], op=OP.mult)
                        if KDBG:
                            nc.sync.dma_start(out=dbg_s[:, 100:228], in_=pooledT[:])
                    if KPHASE >= 9:
                        # MLP
                        h1 = att.tile([P, 1024], f32)
                        for hh in range(2):
                            ph1 = ps.tile([P, 512], f32, tag="ph")
                            for k in range(3):
                                lhs = pooled0[:] if k == 0 else \
                                    pvt2[:, (k - 1) * P:k * P]
                                mm(ph1[:], lhs,
                                   fc1w[:, k * 1024 + hh * 512:
                                        k * 1024 + (hh + 1) * 512],
                                   k == 0, False)
                            mm(ph1[:], ones[:], fc1b[:, hh * 512:(hh + 1) * 512],
                               False, True)
                            nc.scalar.activation(h1[:, hh * 512:(hh + 1) * 512],
                                                 ph1[:], AF.Relu)
                        h1T = att.tile([P, 8 * P], f32)
                        for k in range(8):
                            pt = ps.tile([P, P], f32, tag="s")
                            nc.tensor.transpose(pt[:], h1[:, k * P:(k + 1) * P],
                                                ident[:])
                            nc.vector.tensor_copy(h1T[:, k * P:(k + 1) * P], pt[:])
                        ph2 = ps.tile([P, 512], f32, tag="ph")
                        for k in range(8):
                            mm(ph2[:], h1T[:, k * P:(k + 1) * P],
                               fc2w[:, k * 512:(k + 1) * 512], k == 0, False)
                        mm(ph2[:], ones[:], fc2b[:], False, True)
                        h2 = att.tile([P, 512], f32)
                        nc.scalar.activation(h2[:], ph2[:], AF.Relu)
                        h2T = att.tile([P, 4 * P], f32)
                        for k in range(4):
                            pt = ps.tile([P, P], f32, tag="s")
                            nc.tensor.transpose(pt[:], h2[:, k * P:(k + 1) * P],
                                                ident[:])
                            nc.vector.tensor_copy(h2T[:, k * P:(k + 1) * P], pt[:])
                        po = ps.tile([P, 1], f32, tag="acc")
                        for k in range(4):
                            mm(po[:], h2T[:, k * P:(k + 1) * P], outw[:, k:k + 1],
                               k == 0, False)
                        mm(po[:], ones[:], outb[:], False, True)
                        ysb = wk.tile([P, 1], f32, tag="ysb")
                        nc.vector.tensor_copy(ysb[:], po[:])
                        nc.sync.dma_start(out=y, in_=ysb[:])
                    else:
                        zout2 = wk.tile([P, 1], f32, tag="zout")
                        nc.vector.memset(zout2[:], 0.0)
                        nc.sync.dma_start(out=y, in_=zout2[:])

    nc.compile()
    return nc


_CACHE = {}


def _get_program(meta):
    key = (meta["ttot"], tuple(meta["tiles_per_win"]), KPHASE, KDBG)
    if key not in _CACHE:
        _CACHE[key] = _build_program(meta)
    return _CACHE[key]


def kernel(**inputs) -> np.ndarray:
    from concourse import bass_utils
    in_maps, meta = _host_prep(inputs)
    nc = _get_program(meta)
    res = bass_utils.run_bass_kernel_spmd(nc, in_maps, list(range(NCORES)))
    out = np.zeros((B, 1), np.float32)
    for c in range(NCORES):
        out[c * GPC:(c + 1) * GPC] = res.results[c]["y"]
    return out



# revision 18
# speedup vs baseline: 1.5438x; 1.5438x over previous
"""Bass/Trainium2 kernel for nn_GAT_GCN (GAT -> GCN -> cross-attention -> MLP).

Sharding: 8 cores, each owns 128 consecutive graphs (batch is sorted, so a
contiguous node slab). Edges are assigned to the core owning their dst node,
so all segment reductions are core-local; cross-core data is an AllGather of
(a) small per-node attention tables and (b) the GAT output (needed globally
for the GCN's src-side gather).

Perf structure (v2):
  * fp16 for all heavy matmuls/tables (fp32 matmul = 2 half-rate passes on
    TRN2; fp16 = 1 cycle/row) and for the AllGathered tables (halves fabric
    + HBM traffic).
  * Both AllGathers are chunked (6 x 7 windows) and issued as their windows
    complete, overlapping the collective with compute; outputs live in
    Shared scratchpad (peer-write fast path).
  * Indirect gathers are batched per window (SWDGE fixed cost ~1us/call).

Key algebraic restructurings (verified vs reference):
  * GAT aggregates in x-space (78-wide) instead of h-space (780-wide).
  * Segment sums are matmuls with 0/1 dst-masks built from iota==dstcol.
  * GCN normalization folded: rows pre-scaled by dinv (commutes with relu),
    dinv[dst] folded into the mask values.
  * Softmax max-subtraction dropped (logits are O(1); it cancels exactly).
"""
import os
import sys
import numpy as np

sys.path.insert(0, "/opt/trn_rl_repo")

KPHASE = int(os.environ.get("KPHASE", "9"))
KDBG = os.environ.get("KDBG", "0") == "1"

N, E, B = 40000, 160000, 1024
FXD, H, DOUT, DP = 78, 10, 128, 256
NCORES, GPC = 8, 128          # graphs per core
P = 128
NPAD = 5376                   # padded nodes per core (42 * 128)
NW = NPAD // P                # node windows per core
NTOT = NCORES * NPAD          # padded global node space
HF = H * FXD                  # 780
HW1 = H * (FXD + 1)           # 790, z-psum width
SLOTS = 64                    # pooling slots per graph
KC, NKC = 112, 7              # gcn/fcg1 K-chunk size/count (7*112=784>=780)
SRCW = FXD + 1 + H + 3        # 92: [x | 1 | asrc | pad]
DUMMY0 = NPAD  # pooling pad slots point at dummy -inf rows
NCHUNK, CW = 6, 7             # allgather chunks: 6 chunks x 7 windows
CROWS = CW * P                # 896 rows per core per chunk
CH8 = NCORES * CROWS          # 7168 rows per chunk in the gathered tables
NEG = -60000.0                # -inf stand-in representable in fp16


# ----------------------------------------------------------------------------
# host preprocessing
# ----------------------------------------------------------------------------

def _host_prep(inputs):
    x = np.ascontiguousarray(np.asarray(inputs["x"], dtype=np.float32))
    edge_index = np.asarray(inputs["edge_index"]).astype(np.int64)
    batch = np.asarray(inputs["batch"]).astype(np.int64)
    pvec = np.asarray(inputs["protein_vec"], dtype=np.float32)

    src = np.concatenate([edge_index[0], np.arange(N, dtype=np.int64)])
    dst = np.concatenate([edge_index[1], np.arange(N, dtype=np.int64)])
    order = np.argsort(dst, kind="stable")
    src, dst = src[order], dst[order]

    node_core = batch // GPC
    n0 = np.searchsorted(node_core, np.arange(NCORES))
    n1 = np.searchsorted(node_core, np.arange(NCORES), side="right")
    ncount = n1 - n0
    assert ncount.max() <= NPAD, ncount.max()
    # global id in the chunk-blocked gathered tables:
    # id(core c, local row r) = (r//CROWS)*CH8 + c*CROWS + r%CROWS
    pad_id = np.zeros(N, np.int64)
    for c in range(NCORES):
        r = np.arange(ncount[c])
        pad_id[n0[c]:n1[c]] = (r // CROWS) * CH8 + c * CROWS + (r % CROWS)

    deg = np.bincount(dst, minlength=N).astype(np.float32)

    # per (core, window) edge lists; shared tile schedule = max over cores
    e_start = np.searchsorted(dst, n0)
    e_end = np.searchsorted(dst, n1)
    win_edges = [[None] * NW for _ in range(NCORES)]
    tiles_per_win = np.zeros(NW, np.int64)
    for c in range(NCORES):
        es, ed = src[e_start[c]:e_end[c]], dst[e_start[c]:e_end[c]]
        loc = ed - n0[c]
        wbound = np.searchsorted(loc, np.arange(NW + 1) * P)
        for w in range(NW):
            a, b = int(wbound[w]), int(wbound[w + 1])
            win_edges[c][w] = (es[a:b], loc[a:b])
            tiles_per_win[w] = max(tiles_per_win[w], (b - a + P - 1) // P)
    ttot = int(tiles_per_win.sum())
    tile_start = np.zeros(NW + 1, np.int64)
    for w in range(NW):
        tile_start[w + 1] = tile_start[w] + tiles_per_win[w]

    meta = dict(ttot=ttot, tiles_per_win=[int(v) for v in tiles_per_win],
                tile_start=[int(v) for v in tile_start])

    # weight folds (host: tiny)
    gat_w = np.asarray(inputs["gat_w"], np.float32)
    a_s = np.asarray(inputs["gat_att_src"], np.float32)
    a_d = np.asarray(inputs["gat_att_dst"], np.float32)
    gat_b = np.asarray(inputs["gat_b"], np.float32)
    Vsd = np.zeros((FXD, 2 * H), np.float32)
    Wcat = np.zeros((FXD + 1, HF), np.float32)
    for h in range(H):
        Wh = gat_w[:, h * FXD:(h + 1) * FXD]
        Vsd[:, h] = Wh @ a_s[h]
        Vsd[:, H + h] = Wh @ a_d[h]
        Wcat[:FXD, h * FXD:(h + 1) * FXD] = Wh
        Wcat[FXD, h * FXD:(h + 1) * FXD] = gat_b[h * FXD:(h + 1) * FXD]

    def kchunk(wm, kc, nk, ncols, dt=np.float32):  # [K, ncols] -> [kc, nk*ncols]
        wp = np.zeros((kc * nk, ncols), np.float32)
        wp[:wm.shape[0]] = wm
        out = np.zeros((kc, nk * ncols), np.float32)
        for k in range(nk):
            out[:, k * ncols:(k + 1) * ncols] = wp[k * kc:(k + 1) * kc]
        return out.astype(dt)

    f16 = np.float16
    shared = {
        "v_sd": Vsd.astype(f16), "wcat": Wcat.astype(f16),
        "gcnw": kchunk(np.asarray(inputs["gcn_w"], np.float32), KC, NKC, HF, f16),
        "gcnb": np.asarray(inputs["gcn_b"], f16)[None, :],
        "fcg1w": kchunk(np.asarray(inputs["fcg1_w"], np.float32), KC, NKC, DOUT,
                        f16),
        "fcg1b": np.asarray(inputs["fcg1_b"], np.float32)[:, None],
        "pfcw": kchunk(np.asarray(inputs["pfc_w"], np.float32), P, 2, DP),
        "pfcb": np.asarray(inputs["pfc_b"], np.float32)[None, :],
        "qw": np.asarray(inputs["q_w"], np.float32),
        "qb": np.asarray(inputs["q_b"], np.float32)[None, :],
        "kw": kchunk(np.asarray(inputs["k_w"], np.float32), P, 2, P),
        "kbr": np.asarray(inputs["k_b"], np.float32)[None, :],
        "vw": kchunk(np.asarray(inputs["v_w"], np.float32), P, 2, P),
        "vbr": np.asarray(inputs["v_b"], np.float32)[None, :],
        "fc1w": kchunk(np.asarray(inputs["fc1_w"], np.float32), P, 3, 1024),
        "fc1b": np.asarray(inputs["fc1_b"], np.float32)[None, :],
        "fc2w": kchunk(np.asarray(inputs["fc2_w"], np.float32), P, 8, 512),
        "fc2b": np.asarray(inputs["fc2_b"], np.float32)[None, :],
        "outw": kchunk(np.asarray(inputs["out_w"], np.float32), P, 4, 1),
        "outb": np.asarray(inputs["out_b"], np.float32)[None, :],
        "iota": np.tile(np.arange(P, dtype=np.float32), (P, 1)),
        "ident": np.eye(P, dtype=np.float32),
        "ident_h": np.eye(P, dtype=f16),
    }

    in_maps = []
    for c in range(NCORES):
        nloc = int(ncount[c])
        src_idx = np.zeros((P, ttot), np.int32)
        dstcol = np.full((P, ttot), 999.0, np.float32)
        dinv_e = np.zeros((P, ttot), np.float32)
        for w in range(NW):
            es, loc = win_edges[c][w]
            ne = len(es)
            for j in range(int(tiles_per_win[w])):
                t = int(tile_start[w]) + j
                a, b = j * P, min((j + 1) * P, ne)
                if a >= ne:
                    continue
                m = b - a
                src_idx[:m, t] = pad_id[es[a:b]]
                gdst = loc[a:b] + n0[c]
                dstcol[:m, t] = (loc[a:b] - w * P).astype(np.float32)
                dinv_e[:m, t] = deg[gdst] ** -0.5

        x_slab = np.zeros((NPAD, FXD), np.float16)
        x_slab[:nloc] = x[n0[c]:n1[c]].astype(np.float16)

        # node-major [P, NW] arrays: node w*P+p at [p, w]
        deg_flat = np.ones(NPAD, np.float32)
        deg_flat[:nloc] = deg[n0[c]:n1[c]]
        deg_local = deg_flat.reshape(NW, P).T.copy()
        bc_flat = np.full(NPAD, -1.0, np.float32)
        bc_flat[:nloc] = (batch[n0[c]:n1[c]] - c * GPC).astype(np.float32)
        batchcol = bc_flat.reshape(NW, P).T.copy()

        # pooling slots: flat slot g*SLOTS+s -> dense-local node idx
        flat = np.arange(GPC * SLOTS, dtype=np.int64) % P + DUMMY0
        bl = batch[n0[c]:n1[c]] - c * GPC
        gstart = np.searchsorted(bl, np.arange(GPC + 1))
        for g in range(GPC):
            a, b = int(gstart[g]), int(gstart[g + 1])
            m = min(b - a, SLOTS)
            flat[g * SLOTS:g * SLOTS + m] = np.arange(a, a + m)
        # gather tile tp reads slots [tp*P,(tp+1)*P): idx_pool[p, tp]
        idx_pool = flat.reshape(GPC * SLOTS // P, P).T.astype(np.int32).copy()

        pv_slab = pvec[c * GPC:(c + 1) * GPC, 0, :]          # [128, 256]
        pvT = np.zeros((P, 2 * P), np.float32)
        for k in range(2):
            pvT[:, k * P:(k + 1) * P] = pv_slab[:, k * P:(k + 1) * P].T

        im = {"x_slab": x_slab, "src_idx": src_idx,
              "dstcol": dstcol, "dinv_e": dinv_e, "deg_local": deg_local,
              "batchcol": batchcol, "idx_pool": idx_pool, "pvt": pvT}
        im.update(shared)
        in_maps.append({k: np.ascontiguousarray(v) for k, v in im.items()})

    return in_maps, meta


# ----------------------------------------------------------------------------
# device program
# ----------------------------------------------------------------------------

def _build_program(meta):
    from concourse import bass, bacc, mybir, tile

    f32 = mybir.dt.float32
    f16 = mybir.dt.float16
    i32 = mybir.dt.int32
    AF = mybir.ActivationFunctionType
    OP = mybir.AluOpType
    AX = mybir.AxisListType
    IOA = bass.IndirectOffsetOnAxis

    ttot = meta["ttot"]
    tpw = meta["tiles_per_win"]
    tstart = meta["tile_start"]

    nc = bacc.Bacc("TRN2", target_bir_lowering=False, debug=False,
                   enable_asserts=False, num_devices=NCORES)

    def din(name, shape, dt=f32):
        return nc.dram_tensor(name, list(shape), dt, kind="ExternalInput").ap()

    x_slab = din("x_slab", (NPAD, FXD), f16)
    d_in = {
        "src_idx": din("src_idx", (P, ttot), i32),
        "dstcol": din("dstcol", (P, ttot)),
        "dinv_e": din("dinv_e", (P, ttot)),
        "deg_local": din("deg_local", (P, NW)),
        "batchcol": din("batchcol", (P, NW)),
        "idx_pool": din("idx_pool", (P, SLOTS), i32),
        "pvt": din("pvt", (P, 2 * P)),
        "v_sd": din("v_sd", (FXD, 2 * H), f16),
        "wcat": din("wcat", (FXD + 1, HF), f16),
        "pfcw": din("pfcw", (P, 2 * DP)),
        "pfcb": din("pfcb", (1, DP)),
        "qw": din("qw", (P, P)),
        "qb": din("qb", (1, P)),
        "kw": din("kw", (P, 2 * P)),
        "kbr": din("kbr", (1, P)),
        "vw": din("vw", (P, 2 * P)),
        "vbr": din("vbr", (1, P)),
        "iota": din("iota", (P, P)),
        "ident": din("ident", (P, P)),
        "ident_h": din("ident_h", (P, P), f16),
    }
    gcnw_d = din("gcnw", (KC, NKC * HF), f16)
    gcnb_d = din("gcnb", (1, HF), f16)
    fcg1w_d = din("fcg1w", (KC, NKC * DOUT), f16)
    fcg1b_d = din("fcg1b", (P, 1))
    fc1w_d = din("fc1w", (P, 3 * 1024))
    fc1b_d = din("fc1b", (1, 1024))
    fc2w_d = din("fc2w", (P, 8 * 512))
    fc2b_d = din("fc2b", (1, 512))
    outw_d = din("outw", (P, 4))
    outb_d = din("outb", (1, 1))

    y = nc.dram_tensor("y", [GPC, 1], f32, kind="ExternalOutput").ap()
    if KDBG:
        dbg_h = nc.dram_tensor("dbg_h", [NPAD, HF], f16,
                               kind="ExternalOutput").ap()
        dbg_m = nc.dram_tensor("dbg_m", [P, NPAD], f32,
                               kind="ExternalOutput").ap()
        dbg_s = nc.dram_tensor("dbg_s", [P, 300], f32,
                               kind="ExternalOutput").ap()
        dbg_g = nc.dram_tensor("dbg_g", [P, 2 * HF], f16,
                               kind="ExternalOutput").ap()

    with tile.TileContext(nc) as tc:
      with tc.tile_pool(name="cst", bufs=1) as cst, \
           tc.tile_pool(name="dram", bufs=1, space="DRAM") as dram:

        def cload(pool, ap_):
            t = pool.tile(list(ap_.shape), ap_.dtype, tag=ap_.tensor.name)
            nc.sync.dma_start(out=t[:], in_=ap_)
            return t

        C = {k: cload(cst, v) for k, v in d_in.items()}
        iota, ident, ident_h = C["iota"], C["ident"], C["ident_h"]

        ones = cst.tile([1, P], f32)
        nc.vector.memset(ones[:], 1.0)
        ones_h = cst.tile([1, P], f16)
        nc.vector.memset(ones_h[:], 1.0)
        degs = cst.tile([P, NW], f32)
        dinv_all = cst.tile([P, NW], f32)
        nc.scalar.sqrt(degs[:], C["deg_local"][:])
        nc.vector.reciprocal(dinv_all[:], degs[:])

        # chunked allgather staging + gathered tables (Shared = peer-write)
        ag_src_in = [dram.tile([CROWS, SRCW], f16, tag=f"ags{k}",
                               name=f"ag_src_in{k}")
                     for k in range(NCHUNK)]
        ag_h_in = [dram.tile([CROWS, HF], f16, tag=f"agh{k}",
                             name=f"ag_h_in{k}")
                   for k in range(NCHUNK)]
        table_src = dram.tile([NTOT, SRCW], f16)
        h2s_full = dram.tile([NTOT, HF], f16)
        att_dense = dram.tile([NPAD + P, DOUT], f16)
        adst_all = cst.tile([P, NW * H], f16)

        rg = [list(range(NCORES))]

        def mm(out, lhsT, rhs, start, stop):
            nc.tensor.matmul(out, lhsT, rhs, start=start, stop=stop)

        # ------------------------------------------------------------------
        # Phase A: build [x|1|asrc] tables, chunk-AllGather them
        # ------------------------------------------------------------------
        with tc.tile_pool(name="pha", bufs=3) as wk, \
             tc.tile_pool(name="pha_ps", bufs=2, space="PSUM") as ps:
            for w in range(NW):
                kck = w // CW
                xw = wk.tile([P, FXD], f16, tag="xw")
                nc.sync.dma_start(out=xw[:], in_=x_slab[w * P:(w + 1) * P, :])
                pst = ps.tile([FXD, P], f16, tag="xt")
                nc.tensor.transpose(pst[:], xw[:], ident_h[:])
                xT = wk.tile([FXD, P], f16, tag="xT")
                nc.vector.tensor_copy(xT[:], pst[:])
                pav = ps.tile([P, 2 * H], f32, tag="av")
                mm(pav[:], xT[:], C["v_sd"][:], True, True)
                asb = wk.tile([P, SRCW], f16, tag="asb")
                nc.vector.tensor_copy(asb[:, 0:FXD], xw[:])
                nc.vector.memset(asb[:, FXD:FXD + 1], 1.0)
                nc.vector.tensor_copy(asb[:, FXD + 1:FXD + 1 + H], pav[:, 0:H])
                nc.vector.memset(asb[:, FXD + 1 + H:SRCW], 0.0)
                nc.vector.tensor_copy(adst_all[:, w * H:(w + 1) * H],
                                      pav[:, H:2 * H])
                wl = w - kck * CW
                nc.sync.dma_start(out=ag_src_in[kck][wl * P:(wl + 1) * P, :],
                                  in_=asb[:])
                if wl == CW - 1:
                    nc.gpsimd.collective_compute(
                        "AllGather", OP.bypass, ins=[ag_src_in[kck].opt()],
                        outs=[table_src[kck * CH8:(kck + 1) * CH8, :].opt()],
                        replica_groups=rg)

        # ------------------------------------------------------------------
        # Phase B: GAT
        # ------------------------------------------------------------------
        # last non-empty window of each chunk: the chunk's AllGather fires
        # right after it (empty windows produce no h2sb rows; their table
        # rows are never indexed).
        last_w = {}
        for w in range(NW):
            if tpw[w] > 0:
                last_w[w // CW] = w
        ag_trigger = {w: k for k, w in last_w.items()}
        with tc.tile_pool(name="phb_g", bufs=2) as gp, \
             tc.tile_pool(name="phb_w", bufs=3) as wk, \
             tc.tile_pool(name="phb_z", bufs=2, space="PSUM") as psz, \
             tc.tile_pool(name="phb_t", bufs=2, space="PSUM") as pstp, \
             tc.tile_pool(name="phb_u", bufs=2, space="PSUM") as psu:
            for w in (range(NW) if KPHASE >= 2 else []):
                    kck = w // CW
                    wl = w - kck * CW
                    nt = tpw[w]
                    if nt == 0:
                        continue
                    t0 = tstart[w]
                    pz = psz.tile([P, HW1], f32, tag="pz")
                    for j in range(nt):
                        t = t0 + j
                        xg = gp.tile([P, SRCW], f16, tag="xg")
                        nc.gpsimd.indirect_dma_start(
                            out=xg[:], out_offset=None, in_=table_src[:],
                            in_offset=IOA(ap=C["src_idx"][:, t:t + 1], axis=0))
                        xgf = xg[:]
                        co = 0
                        msk = wk.tile([P, P], f16, tag="msk")
                        nc.vector.tensor_scalar(
                            out=msk[:], in0=iota[:],
                            scalar1=C["dstcol"][:, t:t + 1],
                            scalar2=None, op0=OP.is_equal)
                        pmt = pstp.tile([P, P], f16, tag="pzt")
                        nc.tensor.transpose(pmt[:], msk[:], ident_h[:])
                        mskT = wk.tile([P, P], f16, tag="mskT")
                        nc.scalar.copy(mskT[:], pmt[:])
                        pad_e = psu.tile([P, H], f32, tag="pu")
                        mm(pad_e[:], mskT[:],
                           adst_all[:, w * H:(w + 1) * H], True, True)
                        asrc_f = wk.tile([P, H], f32, tag="asrc_f")
                        nc.vector.tensor_copy(
                            asrc_f[:], xgf[:, co + FXD + 1:co + FXD + 1 + H])
                        al = wk.tile([P, H], f32, tag="al")
                        nc.vector.tensor_tensor(
                            out=al[:], in0=asrc_f[:], in1=pad_e[:], op=OP.add)
                        lr = wk.tile([P, H], f32, tag="lr")
                        nc.vector.scalar_tensor_tensor(
                            out=lr[:], in0=al[:], scalar=0.2, in1=al[:],
                            op0=OP.mult, op1=OP.max)
                        ea = wk.tile([P, H], f16, tag="ea")
                        nc.scalar.activation(ea[:], lr[:], AF.Exp)
                        xgs = wk.tile([P, H, FXD + 1], f16, tag="xgs")
                        nc.vector.tensor_tensor(
                            out=xgs[:],
                            in0=xgf[:, co:co + FXD + 1].unsqueeze(1)
                                .broadcast_to([P, H, FXD + 1]),
                            in1=ea[:].unsqueeze(2)
                                .broadcast_to([P, H, FXD + 1]),
                            op=OP.mult)
                        xf = xgs[:].rearrange("p a b -> p (a b)")
                        mm(pz[:, 0:512], msk[:], xf[:, 0:512],
                           j == 0, j == nt - 1)
                        mm(pz[:, 512:HW1], msk[:], xf[:, 512:HW1],
                           j == 0, j == nt - 1)
                    zsb = wk.tile([P, HW1], f16, tag="zsb")
                    nc.scalar.copy(zsb[:], pz[:])
                    stmp = wk.tile([P, H], f32, tag="stmp")
                    nc.vector.tensor_scalar(
                        out=stmp[:], in0=zsb[:, FXD::FXD + 1], scalar1=1e-30,
                        scalar2=None, op0=OP.add)
                    sinv = wk.tile([P, H], f32, tag="sinv")
                    nc.vector.reciprocal(sinv[:], stmp[:])
                    sc = wk.tile([P, H], f32, tag="sc")
                    nc.vector.tensor_scalar_mul(sc[:], sinv[:],
                                                dinv_all[:, w:w + 1])
                    h2sb = wk.tile([P, HF], f16, tag="h2sb")
                    for h in range(H):
                        pzt = pstp.tile([P, P], f16, tag="pzt")
                        nc.tensor.transpose(
                            pzt[0:FXD + 1, :],
                            zsb[:, h * (FXD + 1):(h + 1) * (FXD + 1)],
                            ident_h[:])
                        zt = wk.tile([FXD + 1, P], f16, tag="zt")
                        nc.vector.tensor_copy(zt[:], pzt[0:FXD + 1, :])
                        pu = psu.tile([P, FXD], f32, tag="pu")
                        mm(pu[:], zt[:], C["wcat"][:, h * FXD:(h + 1) * FXD],
                           True, True)
                        nc.scalar.activation(
                            h2sb[:, h * FXD:(h + 1) * FXD], pu[:], AF.Relu,
                            scale=sc[:, h:h + 1])
                    nc.sync.dma_start(out=ag_h_in[kck][wl * P:(wl + 1) * P, :],
                                      in_=h2sb[:])
                    if KDBG:
                        nc.sync.dma_start(out=dbg_h[w * P:(w + 1) * P, :],
                                          in_=h2sb[:])
                    if ag_trigger.get(w) is not None:
                        nc.gpsimd.collective_compute(
                            "AllGather", OP.bypass,
                            ins=[ag_h_in[kck].opt()],
                            outs=[h2s_full[kck * CH8:(kck + 1) * CH8,
                                           :].opt()],
                            replica_groups=rg)

        # persistent attention-phase buffers
        with tc.tile_pool(name="att", bufs=1) as att:
            dnT = att.tile([P, NPAD], f32)
            e_all = att.tile([P, NW], f32)
            nc.vector.memset(dnT[:], 0.0)
            nc.vector.memset(e_all[:], 0.0)

            # protein path (independent of the graph phases)
            pv_sb = att.tile([P, DP], f32)
            pvt2 = att.tile([P, 2 * P], f32)
            k_sb = att.tile([P, P], f32)
            v_sb = att.tile([P, P], f32)
            with tc.tile_pool(name="prot_ps", bufs=2, space="PSUM") as ps:
                ppv = ps.tile([P, DP], f32, tag="ppv")
                for k in range(2):
                    mm(ppv[:], C["pvt"][:, k * P:(k + 1) * P],
                       C["pfcw"][:, k * DP:(k + 1) * DP], k == 0, False)
                mm(ppv[:], ones[:], C["pfcb"][:], False, True)
                nc.scalar.activation(pv_sb[:], ppv[:], AF.Relu)
                for k in range(2):
                    pt = ps.tile([P, P], f32, tag="pt")
                    nc.tensor.transpose(pt[:], pv_sb[:, k * P:(k + 1) * P],
                                        ident[:])
                    nc.vector.tensor_copy(pvt2[:, k * P:(k + 1) * P], pt[:])
                for dst_t, wname, bname in ((k_sb, "kw", "kbr"),
                                            (v_sb, "vw", "vbr")):
                    pk = ps.tile([P, P], f32, tag="pk")
                    for k in range(2):
                        mm(pk[:], pvt2[:, k * P:(k + 1) * P],
                           C[wname][:, k * P:(k + 1) * P], k == 0, False)
                    mm(pk[:], ones[:], C[bname][:], False, True)
                    nc.vector.tensor_copy(dst_t[:], pk[:])

            # --------------------------------------------------------------
            # Phase C: GCN + fcg1 (writes dnT)
            # --------------------------------------------------------------
            with tc.tile_pool(name="phc_c", bufs=1) as cc, \
                 tc.tile_pool(name="phc_g", bufs=2) as gp, \
                 tc.tile_pool(name="phc_w", bufs=2) as wk, \
                 tc.tile_pool(name="phc_a", bufs=1, space="PSUM") as psa, \
                 tc.tile_pool(name="phc_t", bufs=2, space="PSUM") as pstp, \
                 tc.tile_pool(name="phc_d", bufs=1, space="PSUM") as psd:
                gcnw = cload(cc, gcnw_d)
                gcnb = cload(cc, gcnb_d)
                fcg1w = cload(cc, fcg1w_d)
                fcg1b = cload(cc, fcg1b_d)
                for w in (range(NW) if KPHASE >= 3 else []):
                    nt = tpw[w]
                    if nt == 0:
                        continue
                    t0 = tstart[w]
                    pagg = psa.tile([P, HF], f32, tag="pagg")
                    for j in range(nt):
                        t = t0 + j
                        hg = gp.tile([P, HF], f16, tag="hg")
                        nc.gpsimd.indirect_dma_start(
                            out=hg[:], out_offset=None, in_=h2s_full[:],
                            in_offset=IOA(ap=C["src_idx"][:, t:t + 1], axis=0))
                        hgf = hg[:]
                        if KDBG and w == 0 and j == 0:
                            nc.sync.dma_start(out=dbg_g[:, 0:HF], in_=hgf[:])
                        co = 0
                        mskd = wk.tile([P, P], f16, tag="mskd")
                        nc.vector.tensor_scalar(
                            out=mskd[:], in0=iota[:],
                            scalar1=C["dstcol"][:, t:t + 1],
                            scalar2=C["dinv_e"][:, t:t + 1],
                            op0=OP.is_equal, op1=OP.mult)
                        mm(pagg[:, 0:512], mskd[:], hgf[:, co:co + 512],
                           j == 0, j == nt - 1)
                        mm(pagg[:, 512:HF], mskd[:], hgf[:, co + 512:co + HF],
                           j == 0, j == nt - 1)
                    asb = wk.tile([P, KC * NKC], f16, tag="asb")
                    nc.scalar.copy(asb[:, 0:HF], pagg[:])
                    nc.vector.memset(asb[:, HF:KC * NKC], 0.0)
                    aT = wk.tile([KC, NKC * P], f16, tag="aT")
                    for k in range(NKC):
                        ptr = pstp.tile([KC, P], f16, tag="ptr")
                        nc.tensor.transpose(ptr[:],
                                            asb[:, k * KC:(k + 1) * KC],
                                            ident_h[:])
                        nc.vector.tensor_copy(aT[:, k * P:(k + 1) * P],
                                              ptr[:])
                    pdr = psd.tile([P, HF], f32, tag="pdr")
                    for k in range(NKC):
                        mm(pdr[:, 0:512], aT[:, k * P:(k + 1) * P],
                           gcnw[:, k * HF:k * HF + 512], k == 0, False)
                        mm(pdr[:, 512:HF], aT[:, k * P:(k + 1) * P],
                           gcnw[:, k * HF + 512:(k + 1) * HF],
                           k == 0, False)
                    mm(pdr[:, 0:512], ones_h[:], gcnb[:, 0:512],
                       False, True)
                    mm(pdr[:, 512:HF], ones_h[:], gcnb[:, 512:HF],
                       False, True)
                    drg = wk.tile([P, KC * NKC], f16, tag="drg")
                    nc.scalar.activation(drg[:, 0:HF], pdr[:], AF.Relu)
                    nc.vector.memset(drg[:, HF:KC * NKC], 0.0)
                    drT = wk.tile([KC, NKC * P], f16, tag="drT")
                    for k in range(NKC):
                        ptr = pstp.tile([KC, P], f16, tag="ptr")
                        nc.tensor.transpose(ptr[:],
                                            drg[:, k * KC:(k + 1) * KC],
                                            ident_h[:])
                        nc.vector.tensor_copy(drT[:, k * P:(k + 1) * P],
                                              ptr[:])
                    pdn = psd.tile([P, P], f32, tag="pdn")
                    for k in range(NKC):
                        mm(pdn[:], fcg1w[:, k * P:(k + 1) * P],
                           drT[:, k * P:(k + 1) * P], k == 0, k == NKC - 1)
                    nc.scalar.activation(dnT[:, w * P:(w + 1) * P], pdn[:],
                                         AF.Relu, bias=fcg1b[:])

            # --------------------------------------------------------------
            # Phase D: cross attention + pooling + MLP
            # --------------------------------------------------------------
            if KDBG and KPHASE >= 3:
                nc.sync.dma_start(out=dbg_m, in_=dnT[:])
            with tc.tile_pool(name="phd_c", bufs=1) as cc, \
                 tc.tile_pool(name="phd_w", bufs=3) as wk, \
                 tc.tile_pool(name="phd_ps", bufs=2, space="PSUM") as ps:
                fc1w = cload(cc, fc1w_d)
                fc1b = cload(cc, fc1b_d)
                fc2w = cload(cc, fc2w_d)
                fc2b = cload(cc, fc2b_d)
                outw = cload(cc, outw_d)
                outb = cload(cc, outb_d)

                def build_g01(b):
                    g01t = wk.tile([P, P], f32, tag="g01t")
                    nc.vector.tensor_scalar(
                        out=g01t[:], in0=iota[:],
                        scalar1=C["batchcol"][:, b:b + 1],
                        scalar2=None, op0=OP.is_equal)
                    return g01t

                def build_g01_gmajor(b):
                    g01t = build_g01(b)
                    pg = ps.tile([P, P], f32, tag="s")
                    nc.tensor.transpose(pg[:], g01t[:], ident[:])
                    gsb = wk.tile([P, P], f32, tag="gsb")
                    nc.vector.tensor_copy(gsb[:], pg[:])
                    return gsb

                # pass 1: scores
                for b in (range(NW) if KPHASE >= 4 else []):
                    gsb = build_g01_gmajor(b)
                    pq = ps.tile([P, P], f32, tag="pq")
                    mm(pq[:], dnT[:, b * P:(b + 1) * P], C["qw"][:],
                       True, False)
                    mm(pq[:], ones[:], C["qb"][:], False, True)
                    pkb = ps.tile([P, P], f32, tag="s")
                    mm(pkb[:], gsb[:], k_sb[:], True, True)
                    kbs = wk.tile([P, P], f32, tag="kbs")
                    nc.scalar.copy(kbs[:], pkb[:])
                    qkb = wk.tile([P, P], f32, tag="qkb")
                    nc.vector.tensor_tensor(out=qkb[:], in0=pq[:],
                                            in1=kbs[:], op=OP.mult)
                    scb = wk.tile([P, 1], f32, tag="scb")
                    nc.vector.reduce_sum(out=scb[:], in_=qkb[:], axis=AX.X)
                    nc.scalar.activation(e_all[:, b:b + 1], scb[:], AF.Exp,
                                         scale=float(1.0 / np.sqrt(128.0)))
                if KDBG and KPHASE >= 4:
                    nc.sync.dma_start(out=dbg_s[:, 0:NW], in_=e_all[:])
                if KPHASE < 5:
                    zout = wk.tile([P, 1], f32, tag="zout")
                    nc.vector.memset(zout[:], 0.0)
                    nc.sync.dma_start(out=y, in_=zout[:])
                else:
                    # graph-softmax sums
                    pss = ps.tile([P, 1], f32, tag="acc")
                    for b in (range(NW) if KPHASE >= 4 else []):
                        g01t = build_g01(b)
                        mm(pss[:], g01t[:], e_all[:, b:b + 1], b == 0,
                           b == NW - 1)
                    ssum = wk.tile([P, 1], f32, tag="ssum")
                    nc.vector.tensor_scalar(out=ssum[:], in0=pss[:],
                                            scalar1=1e-30, scalar2=None,
                                            op0=OP.add)
                    sinv_g = att.tile([P, 1], f32)
                    nc.vector.reciprocal(sinv_g[:], ssum[:])
                    ninf = wk.tile([P, DOUT], f16, tag="ninf")
                    nc.vector.memset(ninf[:], NEG)
                    nc.sync.dma_start(out=att_dense[NPAD:NPAD + P, :],
                                      in_=ninf[:])
                    # pass 2: attended -> att_dense
                    for b in range(NW):
                        gsb = build_g01_gmajor(b)
                        psn = ps.tile([P, 1], f32, tag="acc")
                        mm(psn[:], gsb[:], sinv_g[:], True, True)
                        atn = wk.tile([P, 1], f32, tag="atn")
                        nc.vector.tensor_tensor(out=atn[:],
                                                in0=e_all[:, b:b + 1],
                                                in1=psn[:], op=OP.mult)
                        pvb = ps.tile([P, P], f32, tag="s")
                        mm(pvb[:], gsb[:], v_sb[:], True, True)
                        va = wk.tile([P, P], f32, tag="va")
                        nc.vector.tensor_scalar_mul(va[:], pvb[:], atn[:])
                        pdt = ps.tile([P, P], f32, tag="s")
                        nc.tensor.transpose(pdt[:], dnT[:, b * P:(b + 1) * P],
                                            ident[:])
                        attb = wk.tile([P, P], f16, tag="attb")
                        nc.vector.tensor_tensor(out=attb[:], in0=pdt[:],
                                                in1=va[:], op=OP.add)
                        nc.sync.dma_start(out=att_dense[b * P:(b + 1) * P, :],
                                          in_=attb[:])
                    if KPHASE >= 6:
                        # pooling
                        pooledT = att.tile([P, P], f32)
                        NPT = GPC * SLOTS // P      # 64 gather tiles
                        for tp in range(NPT):
                            pgt = wk.tile([P, DOUT], f16, tag="pgt")
                            nc.gpsimd.indirect_dma_start(
                                out=pgt[:], out_offset=None, in_=att_dense[:],
                                in_offset=IOA(
                                    ap=C["idx_pool"][:, tp:tp + 1],
                                    axis=0))
                            ppt = ps.tile([P, P], f16, tag="s")
                            nc.tensor.transpose(ppt[:], pgt[:], ident_h[:])
                            g0 = tp * P // SLOTS
                            nc.vector.reduce_max(
                                out=pooledT[:, g0:g0 + 1],
                                in_=ppt[:, 0:SLOTS], axis=AX.X)
                            nc.vector.reduce_max(
                                out=pooledT[:, g0 + 1:g0 + 2],
                                in_=ppt[:, SLOTS:P], axis=AX.X)
                        pmask = wk.tile([P, P], f32, tag="pmask")
                        nc.vector.tensor_scalar(out=pmask[:], in0=pooledT[:],
                                                scalar1=NEG * 0.5,
                                                scalar2=None,
                                                op0=OP.is_ge)
                        pooled0 = att.tile([P, P], f32)
                        nc.vector.tensor_tensor(out=pooled0[:],
                                                in0=pooledT[:],
                                                in1=pmask[:], op=OP.mult)
                        if KDBG:
                            nc.sync.dma_start(out=dbg_s[:, 100:228],
                                              in_=pooledT[:])
                    if KPHASE >= 9:
                        # MLP
                        h1 = att.tile([P, 1024], f32)
                        for hh in range(2):
                            ph1 = ps.tile([P, 512], f32, tag="ph")
                            for k in range(3):
                                lhs = pooled0[:] if k == 0 else \
                                    pvt2[:, (k - 1) * P:k * P]
                                mm(ph1[:], lhs,
                                   fc1w[:, k * 1024 + hh * 512:
                                        k * 1024 + (hh + 1) * 512],
                                   k == 0, False)
                            mm(ph1[:], ones[:], fc1b[:, hh * 512:(hh + 1) * 512],
                               False, True)
                            nc.scalar.activation(h1[:, hh * 512:(hh + 1) * 512],
                                                 ph1[:], AF.Relu)
                        h1T = att.tile([P, 8 * P], f32)
                        for k in range(8):
                            pt = ps.tile([P, P], f32, tag="s")
                            nc.tensor.transpose(pt[:], h1[:, k * P:(k + 1) * P],
                                                ident[:])
                            nc.vector.tensor_copy(h1T[:, k * P:(k + 1) * P],
                                                  pt[:])
                        ph2 = ps.tile([P, 512], f32, tag="ph")
                        for k in range(8):
                            mm(ph2[:], h1T[:, k * P:(k + 1) * P],
                               fc2w[:, k * 512:(k + 1) * 512], k == 0, False)
                        mm(ph2[:], ones[:], fc2b[:], False, True)
                        h2 = att.tile([P, 512], f32)
                        nc.scalar.activation(h2[:], ph2[:], AF.Relu)
                        h2T = att.tile([P, 4 * P], f32)
                        for k in range(4):
                            pt = ps.tile([P, P], f32, tag="s")
                            nc.tensor.transpose(pt[:], h2[:, k * P:(k + 1) * P],
                                                ident[:])
                            nc.vector.tensor_copy(h2T[:, k * P:(k + 1) * P],
                                                  pt[:])
                        po = ps.tile([P, 1], f32, tag="acc")
                        for k in range(4):
                            mm(po[:], h2T[:, k * P:(k + 1) * P], outw[:, k:k + 1],
                               k == 0, False)
                        mm(po[:], ones[:], outb[:], False, True)
                        ysb = wk.tile([P, 1], f32, tag="ysb")
                        nc.vector.tensor_copy(ysb[:], po[:])
                        nc.sync.dma_start(out=y, in_=ysb[:])
                    else:
                        zout2 = wk.tile([P, 1], f32, tag="zout")
                        nc.vector.memset(zout2[:], 0.0)
                        nc.sync.dma_start(out=y, in_=zout2[:])

    nc.compile()
    return nc


_CACHE = {}


def _get_program(meta):
    key = (meta["ttot"], tuple(meta["tiles_per_win"]), KPHASE, KDBG)
    if key not in _CACHE:
        _CACHE[key] = _build_program(meta)
    return _CACHE[key]


def kernel(**inputs) -> np.ndarray:
    from concourse import bass_utils
    in_maps, meta = _host_prep(inputs)
    nc = _get_program(meta)
    res = bass_utils.run_bass_kernel_spmd(nc, in_maps, list(range(NCORES)))
    out = np.zeros((B, 1), np.float32)
    for c in range(NCORES):
        out[c * GPC:(c + 1) * GPC] = res.results[c]["y"]
    return out
